# revision 21
# baseline (speedup 1.0000x reference)
"""Expert-parallel grouped-GEMM FFN (MoE expert module) for TRN2, 8 NeuronCores.

Problem: xs [16384, 1024] grouped contiguously into 16 experts x 1024 tokens.
Per expert e: y = relu(x @ w1[e].T + b1[e]) @ w2[e].T + b2[e].

Sharding: expert-parallel, 2 experts per core. Each core computes its two
experts' FFN independently; outputs are disjoint row-blocks of the result, so
no collectives are needed.

Precision: weights and activations are bf16 (host-side cast), accumulation and
biases fp32.  l2 relative error ~3e-3, well inside the 2e-2 gate, and bf16
buys: half the HBM traffic (46 MB/core), fast-weight-load (LDWEIGHTS hidden
behind the matmul stream), and half the SBUF footprint so x can be
double-buffered across experts.

Per-core schedule (per expert, all matmuls N=512, 128-contraction):
  - mm1: for each of 32 h-tiles k, two 8-matmul PSUM chains (contraction D),
    ACT evicts relu(acc + b1) -> h[k] bf16 in SBUF.  The second token-half
    chain trails DEFER k-tiles so expert 0's first chains need only the first
    token-half of x.
  - mm2: for each of 8 d-tiles, two interleaved 32-matmul PSUM chains
    (contraction H); ACT evicts acc + b2 -> y bf16 -> DMA out per tile.
    The very last tile splits its chain across two PSUM banks so only a DVE
    merge + two parallel DMA issues remain after the final matmul.
  - Startup: ~80 throwaway matmuls on a zeroed tile keep the PE busy (and the
    HAM clock-gate warm at 2.4 GHz) while the first DMAs land; the startup-
    critical transfers ride one sync-ring in priority order (w1[0], x-nt0
    halves, w1[1..7], x-nt1, w1[8..]) since HBM (~330 GB/s) is the startup
    bottleneck.  x for expert s+1 reuses expert s's buffers (xpool bufs=NN),
    which defers its prefetch until mm1(s) retires — no bandwidth contention.
  - Evictions and y writeback on the ACT/scalar queue (HWDGE); weights and x
    on the sync queue; DVE idle except the final merge.
"""

import numpy as np
import ml_dtypes

import concourse.bacc as bacc
import concourse.mybir as mybir
import concourse.tile as tile
from concourse.bass_utils import run_bass_kernel_spmd

P = 128                 # SBUF partitions / PE array dim
D = 1024                # model dim
H = 4096                # hidden dim
E = 16                  # experts
N_TOK = 16384           # total tokens
N_CORES = 8
E_LOC = E // N_CORES    # experts per core = 2
NE = N_TOK // E         # tokens per expert = 1024
DC = D // P             # 8  (d chunks: mm1 contraction / mm2 output)
HC = H // P             # 32 (h chunks)
NT = 512                # matmul moving free dim (one PSUM bank of fp32)
NN = NE // NT           # 2  (token tiles per expert)
DEFER = 6               # k-tiles the second token-half chain trails by

F32 = mybir.dt.float32
BF16 = mybir.dt.bfloat16
NP_BF16 = ml_dtypes.bfloat16

_CACHE = {}


def _build_nc():
    nc = bacc.Bacc(None, target_bir_lowering=False)

    # Host-tiled layouts (see _prep_in_maps for the exact index maps):
    #   xt  [s, nt, p, c, j]   = x_e[nt*512+j, c*128+p]
    #   w1t [s, k, p, c, j]    = w1[e, k*128+j, c*128+p]
    #   w2t [s, dd, p, k, j]   = w2[e, dd*128+j, k*128+p]
    #   b1r [s, p, k]          = b1[e, k*128+p]
    #   b2r [s, p, dd]         = b2[e, dd*128+p]
    #   yt  [s, p, dd, n]      = y_e[n, dd*128+p]
    xt = nc.dram_tensor("xt", [E_LOC, NN, P, DC, NT], BF16, kind="ExternalInput")
    w1t = nc.dram_tensor("w1t", [E_LOC, HC, P, DC, P], BF16, kind="ExternalInput")
    w2t = nc.dram_tensor("w2t", [E_LOC, DC, P, HC, P], BF16, kind="ExternalInput")
    b1r = nc.dram_tensor("b1r", [E_LOC, P, HC], F32, kind="ExternalInput")
    b2r = nc.dram_tensor("b2r", [E_LOC, P, DC], F32, kind="ExternalInput")
    yt = nc.dram_tensor("yt", [E_LOC, P, DC, NE], BF16, kind="ExternalOutput")

    with tile.TileContext(nc) as tc:
        with (
            tc.tile_pool(name="xpool", bufs=NN) as xpool,
            tc.tile_pool(name="hpool", bufs=HC + 4) as hpool,
            tc.tile_pool(name="w1pool", bufs=12) as w1pool,
            tc.tile_pool(name="w2pool", bufs=3) as w2pool,
            tc.tile_pool(name="ypool", bufs=4) as ypool,
            tc.tile_pool(name="cpool", bufs=2 * E_LOC) as cpool,
            tc.tile_pool(name="wpool", bufs=1) as wpool,
            tc.tile_pool(name="ps1", bufs=3, space="PSUM") as ps1,
            tc.tile_pool(name="psw", bufs=1, space="PSUM") as psw,
            tc.tile_pool(name="ps2", bufs=4, space="PSUM") as ps2,
        ):
            # HAM warm-up: throwaway matmuls on a zeroed scratch tile keep
            # the PE busy while the first x/w1 DMAs land, so the clock gate
            # reaches 8/8 (2.4 GHz) before the real chains start and stays
            # there (any >~2.3us PE idle re-throttles to 1.2 GHz).
            warm = wpool.tile([P, P], BF16)
            nc.gpsimd.memset(warm[:], 0.0)
            warm_acc = psw.tile([P, P], F32)
            for _ in range(72):
                nc.tensor.matmul(warm_acc[:], warm[:], warm[:], start=True, stop=True)

            # Expert 0's first x token-half leads the sync ring: HBM is the
            # startup bottleneck (~330 GB/s total), so x must not compete
            # with the w1 stream.  Everything startup-critical rides one ring
            # in priority order: x-nt0, w1[0..7], x-nt1, w1[8..].
            x_cur = [xpool.tile([P, DC, NT], BF16, name="x_t", tag="x_t")
                     for _ in range(NN)]
            # First-chain deps split across both HWDGE rings so they complete
            # together: x(c0..3) + w1[0] on sync, x(c4..7) leading the scalar
            # ring (its deadline is one half-chain later, and the bias loads
            # must not delay its issue).
            nc.sync.dma_start(out=x_cur[0][:, 0 : DC // 2, :],
                              in_=xt[0, 0, :, 0 : DC // 2, :])
            nc.scalar.dma_start(out=x_cur[0][:, DC // 2 :, :],
                                in_=xt[0, 0, :, DC // 2 :, :])

            for s in range(E_LOC):
                b1_t = cpool.tile([P, HC], F32)
                nc.scalar.dma_start(out=b1_t[:], in_=b1r[s])
                b2_t = cpool.tile([P, DC], F32)
                nc.scalar.dma_start(out=b2_t[:], in_=b2r[s])

                # ---------------- mm1: h = relu(x @ w1.T + b1) ----------------
                h_tiles = []
                w1_tiles = []

                def mm1_chain(k, nt):
                    acc = ps1.tile([P, NT], F32, name="acc", tag="acc")
                    for c in range(DC):
                        nc.tensor.matmul(
                            acc[:],
                            w1_tiles[k][:, c, :],
                            x_cur[nt][:, c, :],
                            start=(c == 0),
                            stop=(c == DC - 1),
                        )
                    nc.scalar.activation(
                        h_tiles[k][:, nt * NT : (nt + 1) * NT],
                        acc[:],
                        mybir.ActivationFunctionType.Relu,
                        bias=b1_t[:, k : k + 1],
                    )

                # First 8 w1 DMAs lead the sync ring; expert 0's second x
                # token-half rides behind them (it isn't needed until chain
                # (0, nt1), and issuing it at t=0 would starve the w1 stream).
                HEAD = 8
                for k in range(HEAD):
                    w1_t = w1pool.tile([P, DC, P], BF16, name="w1_t", tag="w1_t")
                    nc.sync.dma_start(out=w1_t[:], in_=w1t[s, k])
                    w1_tiles.append(w1_t)
                if s == 0:
                    nc.sync.dma_start(out=x_cur[1][:], in_=xt[0, 1])

                for k in range(HC):
                    if k >= HEAD:
                        w1_t = w1pool.tile([P, DC, P], BF16, name="w1_t", tag="w1_t")
                        nc.sync.dma_start(out=w1_t[:], in_=w1t[s, k])
                        w1_tiles.append(w1_t)
                    h_tiles.append(hpool.tile([P, NE], BF16, name="h_t", tag="h_t"))
                    mm1_chain(k, 0)
                    if k >= DEFER:
                        mm1_chain(k - DEFER, 1)
                for k in range(HC - DEFER, HC):
                    mm1_chain(k, 1)

                # Prefetch next expert's x while this expert's mm2 runs.
                if s + 1 < E_LOC:
                    x_next = [xpool.tile([P, DC, NT], BF16, name="x_t", tag="x_t")
                              for _ in range(NN)]
                    for nt in range(NN):
                        nc.scalar.dma_start(out=x_next[nt][:], in_=xt[s + 1, nt])
                    x_cur = x_next

                # ---------------- mm2: y = h @ w2.T + b2 ----------------
                for dd in range(DC):
                    w2_t = w2pool.tile([P, HC, P], BF16)
                    nc.sync.dma_start(out=w2_t[:], in_=w2t[s, dd])
                    # For the very last d-tile, run the two token-half chains
                    # sequentially so the first half's eviction + writeback
                    # overlaps the second half's chain (shorter kernel tail).
                    last = s == E_LOC - 1 and dd == DC - 1
                    acc2s = [ps2.tile([P, NT], F32, name="acc2", tag="acc2")
                             for _ in range(NN)]

                    def mm2_evict(nt):
                        y_tile = ypool.tile([P, NT], BF16)
                        nc.scalar.activation(
                            y_tile[:],
                            acc2s[nt][:],
                            mybir.ActivationFunctionType.Identity,
                            bias=b2_t[:, dd : dd + 1],
                        )
                        nc.scalar.dma_start(
                            out=yt[s, :, dd, nt * NT : (nt + 1) * NT],
                            in_=y_tile[:],
                        )

                    if last:
                        # nt0: plain chain; its eviction + writeback hide under
                        # nt1's chains.  nt1: contraction split across two PSUM
                        # banks — bank A is pre-evicted (no bias) while bank B's
                        # chain still runs, so after the final matmul only one
                        # DVE merge (accB + b2 + yA) and two parallel DMA
                        # issues remain on the critical path.
                        for k in range(HC):
                            nc.tensor.matmul(
                                acc2s[0][:],
                                w2_t[:, k, :],
                                h_tiles[k][:, 0:NT],
                                start=(k == 0),
                                stop=(k == HC - 1),
                            )
                        mm2_evict(0)
                        accB = ps2.tile([P, NT], F32, name="acc2", tag="acc2")
                        for k in range(HC // 2):
                            nc.tensor.matmul(
                                acc2s[1][:],
                                w2_t[:, k, :],
                                h_tiles[k][:, NT : 2 * NT],
                                start=(k == 0),
                                stop=(k == HC // 2 - 1),
                            )
                        yA = ypool.tile([P, NT], BF16)
                        nc.scalar.activation(
                            yA[:],
                            acc2s[1][:],
                            mybir.ActivationFunctionType.Copy,
                            bias=0.0,
                        )
                        for k in range(HC // 2, HC):
                            nc.tensor.matmul(
                                accB[:],
                                w2_t[:, k, :],
                                h_tiles[k][:, NT : 2 * NT],
                                start=(k == HC // 2),
                                stop=(k == HC - 1),
                            )
                        yB = ypool.tile([P, NT], BF16)
                        nc.vector.scalar_tensor_tensor(
                            yB[:],
                            accB[:],
                            b2_t[:, dd : dd + 1],
                            yA[:],
                            op0=mybir.AluOpType.add,
                            op1=mybir.AluOpType.add,
                        )
                        nc.sync.dma_start(
                            out=yt[s, :, dd, NT : NT + NT // 2],
                            in_=yB[:, 0 : NT // 2],
                        )
                        nc.scalar.dma_start(
                            out=yt[s, :, dd, NT + NT // 2 : 2 * NT],
                            in_=yB[:, NT // 2 :],
                        )
                    else:
                        for k in range(HC):
                            for nt in range(NN):
                                nc.tensor.matmul(
                                    acc2s[nt][:],
                                    w2_t[:, k, :],
                                    h_tiles[k][:, nt * NT : (nt + 1) * NT],
                                    start=(k == 0),
                                    stop=(k == HC - 1),
                                )
                        for nt in range(NN):
                            mm2_evict(nt)

    nc.finalize()
    return nc


def _get_nc():
    if "nc" not in _CACHE:
        _CACHE["nc"] = _build_nc()
    return _CACHE["nc"]


def _prep_in_maps(xs, w1, b1, w2, b2):
    xs = np.asarray(xs, dtype=np.float32).astype(NP_BF16)
    w1 = np.asarray(w1, dtype=np.float32).astype(NP_BF16)
    b1 = np.asarray(b1, dtype=np.float32)
    w2 = np.asarray(w2, dtype=np.float32).astype(NP_BF16)
    b2 = np.asarray(b2, dtype=np.float32)

    x3 = xs.reshape(E, NE, D)
    in_maps = []
    for core in range(N_CORES):
        es = [E_LOC * core + s for s in range(E_LOC)]
        # xt[s, nt, p, c, j] = x_e[nt*512+j, c*128+p]
        xt = np.stack(
            [x3[e].T.reshape(DC, P, NN, NT).transpose(2, 1, 0, 3) for e in es]
        )
        # w1t[s, k, p, c, j] = w1[e, k*128+j, c*128+p]
        w1t = np.stack(
            [w1[e].reshape(HC, P, DC, P).transpose(0, 3, 2, 1) for e in es]
        )
        # w2t[s, dd, p, k, j] = w2[e, dd*128+j, k*128+p]
        w2t = np.stack(
            [w2[e].reshape(DC, P, HC, P).transpose(0, 3, 2, 1) for e in es]
        )
        # b1r[s, p, k] = b1[e, k*128+p]
        b1r = np.stack([b1[e].reshape(HC, P).T for e in es])
        b2r = np.stack([b2[e].reshape(DC, P).T for e in es])
        in_maps.append(
            {
                "xt": np.ascontiguousarray(xt),
                "w1t": np.ascontiguousarray(w1t),
                "w2t": np.ascontiguousarray(w2t),
                "b1r": np.ascontiguousarray(b1r),
                "b2r": np.ascontiguousarray(b2r),
            }
        )
    return in_maps


def _gather(results):
    y = np.empty((N_TOK, D), dtype=np.float32)
    for core in range(N_CORES):
        out = results[core]["yt"]  # [E_LOC, P, DC, NE] bf16
        for s in range(E_LOC):
            e = E_LOC * core + s
            # yt[s, p, dd, n] = y_e[n, dd*128+p]
            y[e * NE : (e + 1) * NE] = (
                out[s].transpose(2, 1, 0).reshape(NE, D).astype(np.float32)
            )
    return y


def _run(in_maps, **kwargs):
    nc = _get_nc()
    return run_bass_kernel_spmd(nc, in_maps, core_ids=list(range(N_CORES)), **kwargs)


def kernel(xs, fwd_expert_count, w1, b1, w2, b2):
    # fwd_expert_count is uniform (N_TOK // E per expert) by construction,
    # matching the reference, which also hardcodes the uniform grouping.
    in_maps = _prep_in_maps(xs, w1, b1, w2, b2)
    res = _run(in_maps)
    return _gather(res.results)


# revision 22
# speedup vs baseline: 1.0000x; 1.0000x over previous
"""Expert-parallel grouped-GEMM FFN (MoE expert module) for TRN2, 8 NeuronCores.

Problem: xs [16384, 1024] grouped contiguously into 16 experts x 1024 tokens.
Per expert e: y = relu(x @ w1[e].T + b1[e]) @ w2[e].T + b2[e].

Sharding: expert-parallel, 2 experts per core. Each core computes its two
experts' FFN independently; outputs are disjoint row-blocks of the result, so
no collectives are needed.

Precision: weights and activations are bf16 (host-side cast), accumulation and
biases fp32.  l2 relative error ~3e-3, well inside the 2e-2 gate, and bf16
buys: half the HBM traffic (46 MB/core), fast-weight-load (LDWEIGHTS hidden
behind the matmul stream), and half the SBUF footprint so x can be
double-buffered across experts.

Per-core schedule (per expert, all matmuls N=512, 128-contraction):
  - mm1: for each of 32 h-tiles k, two 8-matmul PSUM chains (contraction D),
    ACT evicts relu(acc + b1) -> h[k] bf16 in SBUF.  The second token-half
    chain trails DEFER k-tiles so expert 0's first chains need only the first
    token-half of x.
  - mm2: for each of 8 d-tiles, two interleaved 32-matmul PSUM chains
    (contraction H); ACT evicts acc + b2 -> y bf16 -> DMA out per tile.
    The very last tile splits its chain across two PSUM banks so only a DVE
    merge + two parallel DMA issues remain after the final matmul.
  - Startup: ~80 throwaway matmuls on a zeroed tile keep the PE busy (and the
    HAM clock-gate warm at 2.4 GHz) while the first DMAs land; the startup-
    critical transfers ride one sync-ring in priority order (w1[0], x-nt0
    halves, w1[1..7], x-nt1, w1[8..]) since HBM (~330 GB/s) is the startup
    bottleneck.  x for expert s+1 reuses expert s's buffers (xpool bufs=NN),
    which defers its prefetch until mm1(s) retires — no bandwidth contention.
  - Evictions and y writeback on the ACT/scalar queue (HWDGE); weights and x
    on the sync queue; DVE idle except the final merge.
"""

import numpy as np
import ml_dtypes

import concourse.bacc as bacc
import concourse.mybir as mybir
import concourse.tile as tile
from concourse.bass_utils import run_bass_kernel_spmd

P = 128                 # SBUF partitions / PE array dim
D = 1024                # model dim
H = 4096                # hidden dim
E = 16                  # experts
N_TOK = 16384           # total tokens
N_CORES = 8
E_LOC = E // N_CORES    # experts per core = 2
NE = N_TOK // E         # tokens per expert = 1024
DC = D // P             # 8  (d chunks: mm1 contraction / mm2 output)
HC = H // P             # 32 (h chunks)
NT = 512                # matmul moving free dim (one PSUM bank of fp32)
NN = NE // NT           # 2  (token tiles per expert)
DEFER = 6               # k-tiles the second token-half chain trails by

F32 = mybir.dt.float32
BF16 = mybir.dt.bfloat16
NP_BF16 = ml_dtypes.bfloat16

_CACHE = {}


def _build_nc():
    nc = bacc.Bacc(None, target_bir_lowering=False)

    # Host-tiled layouts (see _prep_in_maps for the exact index maps):
    #   xt  [s, nt, p, c, j]   = x_e[nt*512+j, c*128+p]
    #   w1t [s, k, p, c, j]    = w1[e, k*128+j, c*128+p]
    #   w2t [s, dd, p, k, j]   = w2[e, dd*128+j, k*128+p]
    #   b1r [s, p, k]          = b1[e, k*128+p]
    #   b2r [s, p, dd]         = b2[e, dd*128+p]
    #   yt  [s, p, dd, n]      = y_e[n, dd*128+p]
    xt = nc.dram_tensor("xt", [E_LOC, NN, P, DC, NT], BF16, kind="ExternalInput")
    w1t = nc.dram_tensor("w1t", [E_LOC, HC, P, DC, P], BF16, kind="ExternalInput")
    w2t = nc.dram_tensor("w2t", [E_LOC, DC, P, HC, P], BF16, kind="ExternalInput")
    b1r = nc.dram_tensor("b1r", [E_LOC, P, HC], F32, kind="ExternalInput")
    b2r = nc.dram_tensor("b2r", [E_LOC, P, DC], F32, kind="ExternalInput")
    yt = nc.dram_tensor("yt", [E_LOC, P, DC, NE], BF16, kind="ExternalOutput")

    with tile.TileContext(nc) as tc:
        with (
            tc.tile_pool(name="xpool", bufs=NN) as xpool,
            tc.tile_pool(name="hpool", bufs=HC + 4) as hpool,
            tc.tile_pool(name="w1pool", bufs=12) as w1pool,
            tc.tile_pool(name="w2pool", bufs=3) as w2pool,
            tc.tile_pool(name="ypool", bufs=4) as ypool,
            tc.tile_pool(name="cpool", bufs=2 * E_LOC) as cpool,
            tc.tile_pool(name="wpool", bufs=1) as wpool,
            tc.tile_pool(name="ps1", bufs=3, space="PSUM") as ps1,
            tc.tile_pool(name="psw", bufs=1, space="PSUM") as psw,
            tc.tile_pool(name="ps2", bufs=4, space="PSUM") as ps2,
        ):
            # HAM warm-up: throwaway matmuls on a zeroed scratch tile keep
            # the PE busy while the first x/w1 DMAs land, so the clock gate
            # reaches 8/8 (2.4 GHz) before the real chains start and stays
            # there (any >~2.3us PE idle re-throttles to 1.2 GHz).
            warm = wpool.tile([P, P], BF16)
            nc.gpsimd.memset(warm[:], 0.0)
            warm_acc = psw.tile([P, P], F32)
            for _ in range(80):
                nc.tensor.matmul(warm_acc[:], warm[:], warm[:], start=True, stop=True)

            # Expert 0's first x token-half leads the sync ring: HBM is the
            # startup bottleneck (~330 GB/s total), so x must not compete
            # with the w1 stream.  Everything startup-critical rides one ring
            # in priority order: x-nt0, w1[0..7], x-nt1, w1[8..].
            x_cur = [xpool.tile([P, DC, NT], BF16, name="x_t", tag="x_t")
                     for _ in range(NN)]

            for s in range(E_LOC):
                b1_t = cpool.tile([P, HC], F32)
                nc.scalar.dma_start(out=b1_t[:], in_=b1r[s])
                b2_t = cpool.tile([P, DC], F32)
                nc.scalar.dma_start(out=b2_t[:], in_=b2r[s])

                # ---------------- mm1: h = relu(x @ w1.T + b1) ----------------
                h_tiles = []
                w1_tiles = []

                def mm1_chain(k, nt):
                    acc = ps1.tile([P, NT], F32, name="acc", tag="acc")
                    for c in range(DC):
                        nc.tensor.matmul(
                            acc[:],
                            w1_tiles[k][:, c, :],
                            x_cur[nt][:, c, :],
                            start=(c == 0),
                            stop=(c == DC - 1),
                        )
                    nc.scalar.activation(
                        h_tiles[k][:, nt * NT : (nt + 1) * NT],
                        acc[:],
                        mybir.ActivationFunctionType.Relu,
                        bias=b1_t[:, k : k + 1],
                    )

                # First 8 w1 DMAs lead the sync ring; expert 0's second x
                # token-half rides behind them (it isn't needed until chain
                # (0, nt1), and issuing it at t=0 would starve the w1 stream).
                HEAD = 8
                for k in range(HEAD):
                    w1_t = w1pool.tile([P, DC, P], BF16, name="w1_t", tag="w1_t")
                    nc.sync.dma_start(out=w1_t[:], in_=w1t[s, k])
                    w1_tiles.append(w1_t)
                    if s == 0 and k == 0:
                        # x rides right behind w1[0]: first-chain deps complete
                        # earliest and their completion receipts pipeline.
                        nc.sync.dma_start(out=x_cur[0][:, 0 : DC // 2, :],
                                          in_=xt[0, 0, :, 0 : DC // 2, :])
                        nc.sync.dma_start(out=x_cur[0][:, DC // 2 :, :],
                                          in_=xt[0, 0, :, DC // 2 :, :])
                if s == 0:
                    nc.sync.dma_start(out=x_cur[1][:], in_=xt[0, 1])

                for k in range(HC):
                    if k >= HEAD:
                        w1_t = w1pool.tile([P, DC, P], BF16, name="w1_t", tag="w1_t")
                        nc.sync.dma_start(out=w1_t[:], in_=w1t[s, k])
                        w1_tiles.append(w1_t)
                    h_tiles.append(hpool.tile([P, NE], BF16, name="h_t", tag="h_t"))
                    mm1_chain(k, 0)
                    if k >= DEFER:
                        mm1_chain(k - DEFER, 1)
                for k in range(HC - DEFER, HC):
                    mm1_chain(k, 1)

                # Prefetch next expert's x while this expert's mm2 runs.
                if s + 1 < E_LOC:
                    x_next = [xpool.tile([P, DC, NT], BF16, name="x_t", tag="x_t")
                              for _ in range(NN)]
                    for nt in range(NN):
                        nc.scalar.dma_start(out=x_next[nt][:], in_=xt[s + 1, nt])
                    x_cur = x_next

                # ---------------- mm2: y = h @ w2.T + b2 ----------------
                for dd in range(DC):
                    w2_t = w2pool.tile([P, HC, P], BF16)
                    nc.sync.dma_start(out=w2_t[:], in_=w2t[s, dd])
                    # For the very last d-tile, run the two token-half chains
                    # sequentially so the first half's eviction + writeback
                    # overlaps the second half's chain (shorter kernel tail).
                    last = s == E_LOC - 1 and dd == DC - 1
                    acc2s = [ps2.tile([P, NT], F32, name="acc2", tag="acc2")
                             for _ in range(NN)]

                    def mm2_evict(nt):
                        y_tile = ypool.tile([P, NT], BF16)
                        nc.scalar.activation(
                            y_tile[:],
                            acc2s[nt][:],
                            mybir.ActivationFunctionType.Identity,
                            bias=b2_t[:, dd : dd + 1],
                        )
                        nc.scalar.dma_start(
                            out=yt[s, :, dd, nt * NT : (nt + 1) * NT],
                            in_=y_tile[:],
                        )

                    if last:
                        # nt0: plain chain; its eviction + writeback hide under
                        # nt1's chains.  nt1: contraction split across two PSUM
                        # banks — bank A is pre-evicted (no bias) while bank B's
                        # chain still runs, so after the final matmul only one
                        # DVE merge (accB + b2 + yA) and two parallel DMA
                        # issues remain on the critical path.
                        for k in range(HC):
                            nc.tensor.matmul(
                                acc2s[0][:],
                                w2_t[:, k, :],
                                h_tiles[k][:, 0:NT],
                                start=(k == 0),
                                stop=(k == HC - 1),
                            )
                        mm2_evict(0)
                        accB = ps2.tile([P, NT], F32, name="acc2", tag="acc2")
                        for k in range(HC // 2):
                            nc.tensor.matmul(
                                acc2s[1][:],
                                w2_t[:, k, :],
                                h_tiles[k][:, NT : 2 * NT],
                                start=(k == 0),
                                stop=(k == HC // 2 - 1),
                            )
                        yA = ypool.tile([P, NT], BF16)
                        nc.scalar.activation(
                            yA[:],
                            acc2s[1][:],
                            mybir.ActivationFunctionType.Copy,
                            bias=0.0,
                        )
                        for k in range(HC // 2, HC):
                            nc.tensor.matmul(
                                accB[:],
                                w2_t[:, k, :],
                                h_tiles[k][:, NT : 2 * NT],
                                start=(k == HC // 2),
                                stop=(k == HC - 1),
                            )
                        yB = ypool.tile([P, NT], BF16)
                        nc.vector.scalar_tensor_tensor(
                            yB[:],
                            accB[:],
                            b2_t[:, dd : dd + 1],
                            yA[:],
                            op0=mybir.AluOpType.add,
                            op1=mybir.AluOpType.add,
                        )
                        nc.sync.dma_start(
                            out=yt[s, :, dd, NT : NT + NT // 2],
                            in_=yB[:, 0 : NT // 2],
                        )
                        nc.scalar.dma_start(
                            out=yt[s, :, dd, NT + NT // 2 : 2 * NT],
                            in_=yB[:, NT // 2 :],
                        )
                    else:
                        for k in range(HC):
                            for nt in range(NN):
                                nc.tensor.matmul(
                                    acc2s[nt][:],
                                    w2_t[:, k, :],
                                    h_tiles[k][:, nt * NT : (nt + 1) * NT],
                                    start=(k == 0),
                                    stop=(k == HC - 1),
                                )
                        for nt in range(NN):
                            mm2_evict(nt)

    nc.finalize()
    return nc


def _get_nc():
    if "nc" not in _CACHE:
        _CACHE["nc"] = _build_nc()
    return _CACHE["nc"]


def _prep_in_maps(xs, w1, b1, w2, b2):
    xs = np.asarray(xs, dtype=np.float32).astype(NP_BF16)
    w1 = np.asarray(w1, dtype=np.float32).astype(NP_BF16)
    b1 = np.asarray(b1, dtype=np.float32)
    w2 = np.asarray(w2, dtype=np.float32).astype(NP_BF16)
    b2 = np.asarray(b2, dtype=np.float32)

    x3 = xs.reshape(E, NE, D)
    in_maps = []
    for core in range(N_CORES):
        es = [E_LOC * core + s for s in range(E_LOC)]
        # xt[s, nt, p, c, j] = x_e[nt*512+j, c*128+p]
        xt = np.stack(
            [x3[e].T.reshape(DC, P, NN, NT).transpose(2, 1, 0, 3) for e in es]
        )
        # w1t[s, k, p, c, j] = w1[e, k*128+j, c*128+p]
        w1t = np.stack(
            [w1[e].reshape(HC, P, DC, P).transpose(0, 3, 2, 1) for e in es]
        )
        # w2t[s, dd, p, k, j] = w2[e, dd*128+j, k*128+p]
        w2t = np.stack(
            [w2[e].reshape(DC, P, HC, P).transpose(0, 3, 2, 1) for e in es]
        )
        # b1r[s, p, k] = b1[e, k*128+p]
        b1r = np.stack([b1[e].reshape(HC, P).T for e in es])
        b2r = np.stack([b2[e].reshape(DC, P).T for e in es])
        in_maps.append(
            {
                "xt": np.ascontiguousarray(xt),
                "w1t": np.ascontiguousarray(w1t),
                "w2t": np.ascontiguousarray(w2t),
                "b1r": np.ascontiguousarray(b1r),
                "b2r": np.ascontiguousarray(b2r),
            }
        )
    return in_maps


def _gather(results):
    y = np.empty((N_TOK, D), dtype=np.float32)
    for core in range(N_CORES):
        out = results[core]["yt"]  # [E_LOC, P, DC, NE] bf16
        for s in range(E_LOC):
            e = E_LOC * core + s
            # yt[s, p, dd, n] = y_e[n, dd*128+p]
            y[e * NE : (e + 1) * NE] = (
                out[s].transpose(2, 1, 0).reshape(NE, D).astype(np.float32)
            )
    return y


def _run(in_maps, **kwargs):
    nc = _get_nc()
    return run_bass_kernel_spmd(nc, in_maps, core_ids=list(range(N_CORES)), **kwargs)


def kernel(xs, fwd_expert_count, w1, b1, w2, b2):
    # fwd_expert_count is uniform (N_TOK // E per expert) by construction,
    # matching the reference, which also hardcodes the uniform grouping.
    in_maps = _prep_in_maps(xs, w1, b1, w2, b2)
    res = _run(in_maps)
    return _gather(res.results)
